# revision 3
# baseline (speedup 1.0000x reference)
"""Trainium2 Bass kernel for nn_ContrastiveMSELoss (8192x8192 cos-sim contrastive + MSE).

Sharding: 8 NeuronCores, users row-sharded 1024/core, full recipe table per core.

Loss decomposition (ratings matrix never materialized):
    rowR[i]  = 0.1*M + sum_{final scatter cells in row i}(v - 0.1)
    S1       = 0.1*T + sum_pairs (v-0.1)*cos[u,i],  T = (sum_i u_i/|u_i|) . (sum_j r_j/|r_j|)
    S2       = sum_i rowR[i] * ln(rowsum_exp[i])
    S3       = sum_i rowR[i] * ln(colsum_exp[i])    (col_sum indexed by i: torch n==m quirk)
    loss     = 0.5*(S2 + S3 - 2*S1)/(2*N) + 0.5*mean((ratings-cos_sim)^2)

Device computes, per core: the 1024x8192 exp(cos) block via fp8e4 DoubleRow
matmuls (host ships pre-normalized, pre-transposed fp8 operands), exp split
across three lanes (ACT exp / Pool and DVE Schraudolph bit-trick exp in fp16),
per-row sums (DVE tensor_scalar accumulate), per-column partial sums (PE
ones-matmul chains into 4 PSUM tiles x 4 tile positions), and the scattered
pair dot products (Pool dma_gather of bf16 rows + DVE dots).

No collective: each core ships its rowsums [1024], colsum partials [8192] and
pair-term partial [128]; the host does the O(N) ln/dot glue (exactly like the
scalar summing it already does), plus the exact closed-form T and MSE terms.
"""

import sys

sys.path.insert(0, "/opt/trn_rl_repo")

import numpy as np
import ml_dtypes

import concourse.bass as bass
import concourse.bacc as bacc
import concourse.tile as tile
from concourse import mybir
from concourse.bass_utils import run_bass_kernel_spmd

f32 = mybir.dt.float32
bf16 = mybir.dt.bfloat16
fp16 = mybir.dt.float16
fp8 = mybir.dt.float8e4
i16 = mybir.dt.int16
AF = mybir.ActivationFunctionType
OP = mybir.AluOpType
AX = mybir.AxisListType
PM = mybir.MatmulPerfMode

NCORES = 8
N = 8192          # users
M = 8192          # recipes
D = 64
B = 65536
S = N // NCORES   # slab rows per core (1024)
RT = S // 128     # row tiles per slab (8)
NG = 8            # column groups of 1024
ALPHA = 0.5
FILL = 0.1
GATHER_CHUNK = 512

# Schraudolph fast-exp in fp16-bit domain: i16 = round(A*x + B), bitcast fp16.
SCH_A = 1024.0 / float(np.log(2.0))   # 1477.32
SCH_B = 15.0 * 1024.0 - 40.0          # c=40 zeroes the exp-sum bias for cos~N(0,1/64)

# exp lane assignment: ACT / POOL / DVE weighted round-robin over the 64 tiles
LANE_W = {"A": 40, "P": 18, "V": 6}


def _lane_list():
    lanes = []
    cnt = {k: 0 for k in LANE_W}
    for i in range(64):
        best, bestv = None, None
        for k, w in LANE_W.items():
            v = w * (i + 1) / 64.0 - cnt[k]
            if bestv is None or v > bestv:
                best, bestv = k, v
        lanes.append(best)
        cnt[best] += 1
    return lanes


def build_nc(K):
    """Build the SPMD Bass program. K = pair slots per partition (128*K pairs/core)."""
    nc = bacc.Bacc(num_devices=NCORES)
    NP = 128 * K

    u8_d = nc.declare_dram_parameter("u8", [32, 2, S], fp8, isOutput=False)
    r8_d = nc.declare_dram_parameter("r8", [32, 2, M], fp8, isOutput=False)
    uhat_d = nc.declare_dram_parameter("uhat", [S, 2 * D], bf16, isOutput=False)
    rhat_d = nc.declare_dram_parameter("rhat", [M, 2 * D], bf16, isOutput=False)
    pair_u = nc.declare_dram_parameter("pair_u", [128, NP // 16], i16, isOutput=False)
    pair_i = nc.declare_dram_parameter("pair_i", [128, NP // 16], i16, isOutput=False)
    pair_w = nc.declare_dram_parameter("pair_w", [128, K], f32, isOutput=False)
    out_cs = nc.declare_dram_parameter("out_cs", [M], f32, isOutput=True)
    out_rs = nc.declare_dram_parameter("out_rs", [128, RT], f32, isOutput=True)
    out_sc = nc.declare_dram_parameter("out_sc", [128, 1], f32, isOutput=True)

    lanes = _lane_list()

    with tile.TileContext(nc) as tc:
        with tc.tile_pool(name="sb", bufs=1) as sb, \
             tc.tile_pool(name="ps", bufs=1, space="PSUM") as ps:
            # ---- input loads ----
            u8s = sb.tile([32, 2, S], fp8)
            nc.sync.dma_start(out=u8s[:], in_=u8_d[:])
            r8g = []
            for g in range(NG):
                t = sb.tile([32, 2, 1024], fp8, name=f"r8g{g}")
                nc.sync.dma_start(out=t[:], in_=r8_d[:, :, g * 1024:(g + 1) * 1024])
                r8g.append(t)
            pu = sb.tile([128, NP // 16], i16)
            nc.sync.dma_start(out=pu[:], in_=pair_u[:])
            pi = sb.tile([128, NP // 16], i16)
            nc.sync.dma_start(out=pi[:], in_=pair_i[:])
            pw = sb.tile([128, K], f32)
            nc.sync.dma_start(out=pw[:], in_=pair_w[:])

            ones16 = sb.tile([128, 1], fp16)
            nc.vector.memset(ones16[:], 1.0)

            # pair gathers (Pool SWDGE; sources are DRAM inputs, can start at once)
            ug = sb.tile([128, K, 2 * D], bf16)
            rg = sb.tile([128, K, 2 * D], bf16)
            for off in range(0, NP, GATHER_CHUNK):
                n = min(GATHER_CHUNK, NP - off)
                nc.gpsimd.dma_gather(
                    ug[:, off // 128:(off + n) // 128, :], uhat_d[:],
                    pu[:, off // 16:(off + n) // 16], n, n, 2 * D)
            for off in range(0, NP, GATHER_CHUNK):
                n = min(GATHER_CHUNK, NP - off)
                nc.gpsimd.dma_gather(
                    rg[:, off // 128:(off + n) // 128, :], rhat_d[:],
                    pi[:, off // 16:(off + n) // 16], n, n, 2 * D)

            # ---- colsum PSUM tiles: tile t holds chains for groups 2t, 2t+1
            # at partition offsets (g-even half0)0,(h1)32,(g-odd h0)64,(h1)96
            cs_tiles = [ps.tile([128, 512], f32, name=f"cs{t}") for t in range(4)]

            rs_acc = sb.tile([128, NG * RT], f32)
            junk = sb.tile([128, 1024], fp16)

            # ---- main loop ----
            li = 0
            for g in range(NG):
                cs_t = cs_tiles[g // 2]
                for r in range(RT):
                    pg = ps.tile([128, 1024], f32, tag="cos", bufs=2)
                    for jj in range(2):
                        nc.tensor.matmul(
                            out=pg[:, jj * 512:(jj + 1) * 512],
                            lhsT=u8s[:, :, r * 128:(r + 1) * 128],
                            rhs=r8g[g][:, :, jj * 512:(jj + 1) * 512],
                            start=True, stop=True,
                            perf_mode=PM.DoubleRow)
                    lane = lanes[li]
                    li += 1
                    if lane == "A":
                        ex_t = sb.tile([128, 1024], fp16, tag="ex", bufs=3)
                        nc.scalar.activation(out=ex_t[:], in_=pg[:], func=AF.Exp)
                        ex = ex_t[:]
                    else:
                        eng = nc.gpsimd if lane == "P" else nc.vector
                        tp = sb.tile([128, 1024], fp16, tag=f"sch_t{lane}", bufs=2)
                        eng.tensor_scalar(out=tp[:], in0=pg[:], scalar1=SCH_A,
                                          scalar2=SCH_B, op0=OP.mult, op1=OP.add)
                        ip = sb.tile([128, 1024], i16, tag="ex8", bufs=3)
                        eng.tensor_copy(out=ip[:], in_=tp[:])
                        ex = ip[:].bitcast(fp16)
                    # rowsum partial (DVE TSP accumulate)
                    nc.vector.tensor_scalar(
                        out=junk[:], in0=ex, scalar1=1.0, scalar2=None,
                        op0=OP.mult, op1=OP.add,
                        accum_out=rs_acc[:, g * RT + r:g * RT + r + 1])
                    # colsum chains (PE ones-matmul, 4 positions per PSUM tile)
                    for jj in range(2):
                        pos = (g % 2) * 64 + jj * 32
                        nc.tensor.matmul(
                            out=cs_t[pos:pos + 1, :],
                            lhsT=ones16[:, 0:1],
                            rhs=ex[:, jj * 512:(jj + 1) * 512],
                            start=(r == 0), stop=(r == RT - 1),
                            tile_position=(0, pos),
                            skip_group_check=True)
                # after odd group: finalize the cs tile (copy 4 rows, ship)
                if g % 2 == 1:
                    t = g // 2
                    csb = sb.tile([128, 512], f32, tag="csb", bufs=2)
                    nc.vector.tensor_copy(out=csb[0:128:32, :], in_=cs_tiles[t][0:128:32, :])
                    for q in range(4):
                        seg = (2 * t + q // 2) * 1024 + (q % 2) * 512
                        nc.sync.dma_start(out=out_cs[seg:seg + 512], in_=csb[32 * q:32 * q + 1, :])
                # interleave the pair-dot block mid-program
                if g == 4:
                    x_t = sb.tile([128, K, D], bf16)
                    nc.vector.tensor_tensor(
                        out=x_t[:], in0=ug[:, :, 0:D], in1=rg[:, :, 0:D], op=OP.mult)
                    cosg = sb.tile([128, K], f32)
                    nc.vector.tensor_reduce(out=cosg[:], in_=x_t[:], axis=AX.X, op=OP.add)
                    junk_k = sb.tile([128, K], f32)
                    pair_acc = sb.tile([128, 1], f32)
                    nc.vector.tensor_tensor_reduce(
                        out=junk_k[:], in0=cosg[:], in1=pw[:], scale=1.0, scalar=0.0,
                        op0=OP.mult, op1=OP.add, accum_out=pair_acc[:])
                    nc.sync.dma_start(out=out_sc[:], in_=pair_acc[:])

            # ---- tail: rowsums ----
            rs_f = sb.tile([128, RT], f32)
            nc.vector.tensor_reduce(
                out=rs_f[:],
                in_=rs_acc[:].rearrange("p (g r) -> p r g", g=NG),
                axis=AX.X, op=OP.add)
            nc.sync.dma_start(out=out_rs[:], in_=rs_f[:])
    nc.finalize()
    return nc


def _host_prep(inputs):
    """Normalize embeddings, build fp8/bf16 device operands, dedup+shard pairs."""
    U = np.asarray(inputs["user_embeddings"], dtype=np.float32)
    R = np.asarray(inputs["recipe_embeddings"], dtype=np.float32)
    rat = np.asarray(inputs["ratings_scaled"], dtype=np.float32)
    u = np.asarray(inputs["u_idx"]).astype(np.int64)
    i = np.asarray(inputs["i_idx"]).astype(np.int64)

    un = np.linalg.norm(U.astype(np.float64), axis=1)
    rn = np.linalg.norm(R.astype(np.float64), axis=1)
    uhat = (U / un[:, None]).astype(np.float32)
    rhat = (R / rn[:, None]).astype(np.float32)

    # fp8 transposed layouts: [d(32), t(2), row] with k = 32*t + d
    f8 = ml_dtypes.float8_e4m3
    u8_full = np.stack([uhat[:, 0:32].T, uhat[:, 32:64].T], axis=1).astype(f8)  # [32,2,N]
    r8 = np.ascontiguousarray(np.stack([rhat[:, 0:32].T, rhat[:, 32:64].T], axis=1).astype(f8))

    # bf16 row-major padded to 128 cols (256B rows for dma_gather)
    uhat_pad = np.zeros((N, 2 * D), dtype=ml_dtypes.bfloat16)
    uhat_pad[:, 0:D] = uhat.astype(ml_dtypes.bfloat16)
    rhat_pad = np.zeros((M, 2 * D), dtype=ml_dtypes.bfloat16)
    rhat_pad[:, 0:D] = rhat.astype(ml_dtypes.bfloat16)

    # dedup scatter: last write wins
    cell = u * M + i
    _, idx_rev = np.unique(cell[::-1], return_index=True)
    keep = (B - 1 - idx_rev)
    uu = u[keep].astype(np.int64)
    ii = i[keep].astype(np.int64)
    ww = (rat[keep].astype(np.float64) - FILL)

    delta = np.bincount(uu, weights=ww, minlength=N)
    row_r = (FILL * M + delta)  # fp64 [N]

    core_of = uu // S
    counts = np.bincount(core_of, minlength=NCORES)
    K = max(1, int(np.ceil(counts.max() / 128)))
    cap = 128 * K

    in_maps = []
    for c in range(NCORES):
        sel = core_of == c
        n_c = int(sel.sum())
        pu = np.zeros(cap, dtype=np.int16)
        piv = np.zeros(cap, dtype=np.int16)
        pwv = np.zeros(cap, dtype=np.float32)
        pu[:n_c] = (uu[sel] - c * S).astype(np.int16)
        piv[:n_c] = ii[sel].astype(np.int16)
        pwv[:n_c] = ww[sel].astype(np.float32)
        # dma_gather idx layout: [128, cap//16], row p slot s = idx[s*16 + p%16], tiled 8x
        pu_dev = np.ascontiguousarray(np.tile(pu.reshape(cap // 16, 16).T, (8, 1)))
        pi_dev = np.ascontiguousarray(np.tile(piv.reshape(cap // 16, 16).T, (8, 1)))
        # pair weights in TSP layout [128, K]: slot k of partition p = pair 128*k + p
        pw_dev = np.ascontiguousarray(pwv.reshape(K, 128).T)
        in_maps.append({
            "u8": np.ascontiguousarray(u8_full[:, :, c * S:(c + 1) * S]),
            "r8": r8,
            "uhat": np.ascontiguousarray(uhat_pad[c * S:(c + 1) * S]),
            "rhat": rhat_pad,
            "pair_u": pu_dev,
            "pair_i": pi_dev,
            "pair_w": pw_dev,
        })

    host = {
        "row_r": row_r,
        "T": float(uhat.astype(np.float64).sum(0) @ rhat.astype(np.float64).sum(0)),
        "mse": float(np.mean((rat.astype(np.float64)
                              - np.asarray(inputs["cos_similarities_scaled"], dtype=np.float64)) ** 2)),
    }
    return in_maps, K, host


def kernel(user_embeddings, recipe_embeddings, ratings_scaled, cos_similarities_scaled,
           u_idx, i_idx, _trace=False):
    inputs = {
        "user_embeddings": user_embeddings,
        "recipe_embeddings": recipe_embeddings,
        "ratings_scaled": ratings_scaled,
        "cos_similarities_scaled": cos_similarities_scaled,
        "u_idx": u_idx,
        "i_idx": i_idx,
    }
    in_maps, K, host = _host_prep(inputs)
    nc = build_nc(K)
    res = run_bass_kernel_spmd(nc, in_maps, core_ids=list(range(NCORES)), trace=_trace)
    loss = _combine([res.results[c] for c in range(NCORES)], host)
    if _trace:
        kernel._last_results = res
    return np.float32(loss)


def _combine(outs_per_core, host):
    """Host-side glue: sum colsum partials, ln+dot with rowR, add exact T/MSE."""
    row_r = host["row_r"]  # fp64 [N]
    colsum = np.zeros(M, dtype=np.float64)
    S2 = 0.0
    PAIR = 0.0
    for c in range(NCORES):
        o = outs_per_core[c]
        colsum += np.asarray(o["out_cs"], dtype=np.float64)
        rs = np.asarray(o["out_rs"], dtype=np.float64)      # [128, RT]: user r*128+p
        rows = rs.T.reshape(-1)                              # user index within slab
        S2 += float(row_r[c * S:(c + 1) * S] @ np.log(rows))
        PAIR += float(np.asarray(o["out_sc"], dtype=np.float64).sum())
    S3 = float(row_r @ np.log(colsum))
    S1 = FILL * host["T"] + PAIR
    contrastive = (S2 + S3 - 2.0 * S1) / (2.0 * N)
    return ALPHA * contrastive + (1.0 - ALPHA) * host["mse"]


# revision 7
# speedup vs baseline: 1.4558x; 1.4558x over previous
"""Trainium2 Bass kernel for nn_ContrastiveMSELoss (8192x8192 cos-sim contrastive + MSE).

Sharding: 8 NeuronCores, users row-sharded 1024/core, full recipe table per core.

Loss decomposition (ratings matrix never materialized):
    rowR[i]  = 0.1*M + sum_{final scatter cells in row i}(v - 0.1)
    S1       = 0.1*T + sum_pairs (v-0.1)*cos[u,i],  T = (sum_i u_i/|u_i|) . (sum_j r_j/|r_j|)
    S2       = sum_i rowR[i] * ln(rowsum_exp[i])
    S3       = sum_i rowR[i] * ln(colsum_exp[i])    (col_sum indexed by i: torch n==m quirk)
    loss     = 0.5*(S2 + S3 - 2*S1)/(2*N) + 0.5*mean((ratings-cos_sim)^2)

Device computes, per core: the 1024x8192 exp(cos) block via fp8e4 DoubleRow
matmuls (host ships pre-normalized, pre-transposed fp8 operands), exp split
across three lanes (ACT exp / Pool and DVE Schraudolph bit-trick exp in fp16),
per-row sums (DVE tensor_scalar accumulate), per-column partial sums (PE
ones-matmul chains into 4 PSUM tiles x 4 tile positions), and the scattered
pair dot products (Pool dma_gather of bf16 rows + DVE dots).

No collective: each core ships its rowsums [1024], colsum partials [8192] and
pair-term partial [128]; the host does the O(N) ln/dot glue (exactly like the
scalar summing it already does), plus the exact closed-form T and MSE terms.
"""

import sys

sys.path.insert(0, "/opt/trn_rl_repo")

import numpy as np
import ml_dtypes

import concourse.bass as bass
import concourse.bacc as bacc
import concourse.tile as tile
from concourse import mybir
from concourse.bass_utils import run_bass_kernel_spmd

f32 = mybir.dt.float32
bf16 = mybir.dt.bfloat16
fp16 = mybir.dt.float16
fp8 = mybir.dt.float8e4
i16 = mybir.dt.int16
AF = mybir.ActivationFunctionType
OP = mybir.AluOpType
AX = mybir.AxisListType
PM = mybir.MatmulPerfMode

NCORES = 8
N = 8192          # users
M = 8192          # recipes
D = 64
B = 65536
S = N // NCORES   # slab rows per core (1024)
RT = S // 128     # row tiles per slab (8)
NG = 8            # column groups of 1024
ALPHA = 0.5
FILL = 0.1
GATHER_CHUNK = 512

# Schraudolph fast-exp in fp16-bit domain: i16 = round(A*x + B), bitcast fp16.
SCH_A = 1024.0 / float(np.log(2.0))   # 1477.32
SCH_B = 15.0 * 1024.0 - 40.0          # c=40 zeroes the exp-sum bias for cos~N(0,1/64)

# exp lane assignment: ACT / POOL / DVE weighted round-robin over the 64 tiles
LANE_W = {"A": 40, "P": 18, "V": 6}


def _lane_list():
    lanes = []
    cnt = {k: 0 for k in LANE_W}
    for i in range(64):
        best, bestv = None, None
        for k, w in LANE_W.items():
            v = w * (i + 1) / 64.0 - cnt[k]
            if bestv is None or v > bestv:
                best, bestv = k, v
        lanes.append(best)
        cnt[best] += 1
    return lanes


def build_nc(K):
    """Build the SPMD Bass program. K = pair slots per partition (128*K pairs/core)."""
    nc = bacc.Bacc(num_devices=NCORES)
    NP = 128 * K

    u8_d = nc.declare_dram_parameter("u8", [32, 2, S], fp8, isOutput=False)
    r8_d = nc.declare_dram_parameter("r8", [32, 2, M], fp8, isOutput=False)
    uhat_d = nc.declare_dram_parameter("uhat", [S, 2 * D], bf16, isOutput=False)
    rhat_d = nc.declare_dram_parameter("rhat", [M, 2 * D], bf16, isOutput=False)
    pair_u = nc.declare_dram_parameter("pair_u", [128, NP // 16], i16, isOutput=False)
    pair_i = nc.declare_dram_parameter("pair_i", [128, NP // 16], i16, isOutput=False)
    pair_w = nc.declare_dram_parameter("pair_w", [128, K], f32, isOutput=False)
    out_cs = nc.declare_dram_parameter("out_cs", [M], f32, isOutput=True)
    out_rs = nc.declare_dram_parameter("out_rs", [128, RT], f32, isOutput=True)
    out_sc = nc.declare_dram_parameter("out_sc", [128, 1], f32, isOutput=True)

    lanes = _lane_list()

    with tile.TileContext(nc) as tc:
        with tc.tile_pool(name="sb", bufs=1) as sb, \
             tc.tile_pool(name="ps", bufs=1, space="PSUM") as ps:
            # ---- input loads ----
            u8s = sb.tile([32, 2, S], fp8)
            nc.sync.dma_start(out=u8s[:], in_=u8_d[:])
            r8g = []
            for g in range(NG):
                t = sb.tile([32, 2, 1024], fp8, name=f"r8g{g}")
                nc.sync.dma_start(out=t[:], in_=r8_d[:, :, g * 1024:(g + 1) * 1024])
                r8g.append(t)
            pu = sb.tile([128, NP // 16], i16)
            nc.sync.dma_start(out=pu[:], in_=pair_u[:])
            pi = sb.tile([128, NP // 16], i16)
            nc.sync.dma_start(out=pi[:], in_=pair_i[:])
            pw = sb.tile([128, K], f32)
            nc.sync.dma_start(out=pw[:], in_=pair_w[:])

            ones16 = sb.tile([128, 1], fp16)
            nc.vector.memset(ones16[:], 1.0)

            # pair gathers (Pool SWDGE): issued in chunks interleaved into the
            # main loop so they don't monopolize the in-order Pool queue.
            ug = sb.tile([128, K, 2 * D], bf16)
            rg = sb.tile([128, K, 2 * D], bf16)
            n_chunks = (NP + GATHER_CHUNK - 1) // GATHER_CHUNK  # per tensor

            def issue_gathers(tensor_i, lo, hi):
                dst, src, idx = [(ug, uhat_d, pu), (rg, rhat_d, pi)][tensor_i]
                for ci in range(lo, min(hi, n_chunks)):
                    off = ci * GATHER_CHUNK
                    n = min(GATHER_CHUNK, NP - off)
                    nc.gpsimd.dma_gather(
                        dst[:, off // 128:(off + n) // 128, :], src[:],
                        idx[:, off // 16:(off + n) // 16], n, n, 2 * D)

            rs_acc = sb.tile([128, NG * RT], f32)
            junk = sb.tile([128, 1024], fp16)

            # pair-dot block state (issued in 4 chunks across late groups)
            KC = (K + 3) // 4
            cosg = sb.tile([128, K], f32)
            pair_acc = sb.tile([128, 4], f32)
            nc.vector.memset(pair_acc[:], 0.0)

            def issue_pair_chunk(c):
                ks = slice(c * KC, min((c + 1) * KC, K))
                nk = ks.stop - ks.start
                if nk <= 0:
                    return
                x_t = sb.tile([128, KC, D], bf16, tag="pairx", bufs=2)
                nc.vector.tensor_tensor(
                    out=x_t[:, 0:nk, :], in0=ug[:, ks, 0:D], in1=rg[:, ks, 0:D], op=OP.mult)
                nc.vector.tensor_reduce(
                    out=cosg[:, ks], in_=x_t[:, 0:nk, :], axis=AX.X, op=OP.add)
                junk_k = sb.tile([128, KC], f32, tag="pairj", bufs=2)
                nc.vector.tensor_tensor_reduce(
                    out=junk_k[:, 0:nk], in0=cosg[:, ks], in1=pw[:, ks], scale=1.0,
                    scalar=0.0, op0=OP.mult, op1=OP.add,
                    accum_out=pair_acc[:, c:c + 1])

            # ---- main loop ----
            li = 0
            for g in range(NG):
                if g % 2 == 0:
                    cs_t = ps.tile([128, 512], f32, tag="cs", bufs=2)
                for r in range(RT):
                    pg = ps.tile([128, 1024], f32, tag="cos", bufs=3)
                    for jj in range(2):
                        nc.tensor.matmul(
                            out=pg[:, jj * 512:(jj + 1) * 512],
                            lhsT=u8s[:, :, r * 128:(r + 1) * 128],
                            rhs=r8g[g][:, :, jj * 512:(jj + 1) * 512],
                            start=True, stop=True,
                            perf_mode=PM.DoubleRow)
                    lane = lanes[li]
                    li += 1
                    if lane == "A":
                        ex_t = sb.tile([128, 1024], fp16, tag="ex", bufs=4)
                        nc.scalar.activation(out=ex_t[:], in_=pg[:], func=AF.Exp)
                        ex = ex_t[:]
                    else:
                        eng = nc.gpsimd if lane == "P" else nc.vector
                        tp = sb.tile([128, 1024], fp16, tag=f"sch_t{lane}", bufs=2)
                        eng.tensor_scalar(out=tp[:], in0=pg[:], scalar1=SCH_A,
                                          scalar2=SCH_B, op0=OP.mult, op1=OP.add)
                        ip = sb.tile([128, 1024], i16, tag="ex8", bufs=4)
                        eng.tensor_copy(out=ip[:], in_=tp[:])
                        ex = ip[:].bitcast(fp16)
                    # rowsum partial (DVE TSP accumulate)
                    nc.vector.tensor_scalar(
                        out=junk[:], in0=ex, scalar1=1.0, scalar2=None,
                        op0=OP.mult, op1=OP.add,
                        accum_out=rs_acc[:, g * RT + r:g * RT + r + 1])
                    # colsum chains (PE ones-matmul, 4 positions per PSUM tile)
                    for jj in range(2):
                        pos = (g % 2) * 64 + jj * 32
                        nc.tensor.matmul(
                            out=cs_t[pos:pos + 1, :],
                            lhsT=ones16[:, 0:1],
                            rhs=ex[:, jj * 512:(jj + 1) * 512],
                            start=(r == 0), stop=(r == RT - 1),
                            tile_position=(0, pos),
                            skip_group_check=True)
                # after odd group: finalize the cs tile (copy 4 rows, ship)
                if g % 2 == 1:
                    t = g // 2
                    csb = sb.tile([128, 512], f32, tag="csb", bufs=2)
                    nc.vector.tensor_copy(out=csb[0:128:32, :], in_=cs_t[0:128:32, :])
                    for q in range(4):
                        seg = (2 * t + q // 2) * 1024 + (q % 2) * 512
                        nc.sync.dma_start(out=out_cs[seg:seg + 512], in_=csb[32 * q:32 * q + 1, :])
                # interleave gathers (6 chunks/group keeps Pool fed but not clogged)
                if g < 5:
                    lo = g * 3
                    issue_gathers(0, lo, lo + 3)
                    issue_gathers(1, lo, lo + 3)
                elif g == 5:
                    issue_gathers(0, 15, n_chunks)
                    issue_gathers(1, 15, n_chunks)
                # pair-dot chunks on DVE, spread across late groups
                if 4 <= g <= 7:
                    issue_pair_chunk(g - 4)

            # pair partial -> out
            pair_f = sb.tile([128, 1], f32)
            nc.vector.tensor_reduce(out=pair_f[:], in_=pair_acc[:], axis=AX.X, op=OP.add)
            nc.sync.dma_start(out=out_sc[:], in_=pair_f[:])

            # ---- tail: rowsums ----
            rs_f = sb.tile([128, RT], f32)
            nc.vector.tensor_reduce(
                out=rs_f[:],
                in_=rs_acc[:].rearrange("p (g r) -> p r g", g=NG),
                axis=AX.X, op=OP.add)
            nc.sync.dma_start(out=out_rs[:], in_=rs_f[:])
    nc.finalize()
    return nc


def _host_prep(inputs):
    """Normalize embeddings, build fp8/bf16 device operands, dedup+shard pairs."""
    U = np.asarray(inputs["user_embeddings"], dtype=np.float32)
    R = np.asarray(inputs["recipe_embeddings"], dtype=np.float32)
    rat = np.asarray(inputs["ratings_scaled"], dtype=np.float32)
    u = np.asarray(inputs["u_idx"]).astype(np.int64)
    i = np.asarray(inputs["i_idx"]).astype(np.int64)

    un = np.linalg.norm(U.astype(np.float64), axis=1)
    rn = np.linalg.norm(R.astype(np.float64), axis=1)
    uhat = (U / un[:, None]).astype(np.float32)
    rhat = (R / rn[:, None]).astype(np.float32)

    # fp8 transposed layouts: [d(32), t(2), row] with k = 32*t + d
    f8 = ml_dtypes.float8_e4m3
    u8_full = np.stack([uhat[:, 0:32].T, uhat[:, 32:64].T], axis=1).astype(f8)  # [32,2,N]
    r8 = np.ascontiguousarray(np.stack([rhat[:, 0:32].T, rhat[:, 32:64].T], axis=1).astype(f8))

    # bf16 row-major padded to 128 cols (256B rows for dma_gather)
    uhat_pad = np.zeros((N, 2 * D), dtype=ml_dtypes.bfloat16)
    uhat_pad[:, 0:D] = uhat.astype(ml_dtypes.bfloat16)
    rhat_pad = np.zeros((M, 2 * D), dtype=ml_dtypes.bfloat16)
    rhat_pad[:, 0:D] = rhat.astype(ml_dtypes.bfloat16)

    # dedup scatter: last write wins
    cell = u * M + i
    _, idx_rev = np.unique(cell[::-1], return_index=True)
    keep = (B - 1 - idx_rev)
    uu = u[keep].astype(np.int64)
    ii = i[keep].astype(np.int64)
    ww = (rat[keep].astype(np.float64) - FILL)

    delta = np.bincount(uu, weights=ww, minlength=N)
    row_r = (FILL * M + delta)  # fp64 [N]

    core_of = uu // S
    counts = np.bincount(core_of, minlength=NCORES)
    K = max(1, int(np.ceil(counts.max() / 128)))
    cap = 128 * K

    in_maps = []
    for c in range(NCORES):
        sel = core_of == c
        n_c = int(sel.sum())
        pu = np.zeros(cap, dtype=np.int16)
        piv = np.zeros(cap, dtype=np.int16)
        pwv = np.zeros(cap, dtype=np.float32)
        pu[:n_c] = (uu[sel] - c * S).astype(np.int16)
        piv[:n_c] = ii[sel].astype(np.int16)
        pwv[:n_c] = ww[sel].astype(np.float32)
        # dma_gather idx layout: [128, cap//16], row p slot s = idx[s*16 + p%16], tiled 8x
        pu_dev = np.ascontiguousarray(np.tile(pu.reshape(cap // 16, 16).T, (8, 1)))
        pi_dev = np.ascontiguousarray(np.tile(piv.reshape(cap // 16, 16).T, (8, 1)))
        # pair weights in TSP layout [128, K]: slot k of partition p = pair 128*k + p
        pw_dev = np.ascontiguousarray(pwv.reshape(K, 128).T)
        in_maps.append({
            "u8": np.ascontiguousarray(u8_full[:, :, c * S:(c + 1) * S]),
            "r8": r8,
            "uhat": np.ascontiguousarray(uhat_pad[c * S:(c + 1) * S]),
            "rhat": rhat_pad,
            "pair_u": pu_dev,
            "pair_i": pi_dev,
            "pair_w": pw_dev,
        })

    host = {
        "row_r": row_r,
        "T": float(uhat.astype(np.float64).sum(0) @ rhat.astype(np.float64).sum(0)),
        "mse": float(np.mean((rat.astype(np.float64)
                              - np.asarray(inputs["cos_similarities_scaled"], dtype=np.float64)) ** 2)),
    }
    return in_maps, K, host


def kernel(user_embeddings, recipe_embeddings, ratings_scaled, cos_similarities_scaled,
           u_idx, i_idx, _trace=False):
    inputs = {
        "user_embeddings": user_embeddings,
        "recipe_embeddings": recipe_embeddings,
        "ratings_scaled": ratings_scaled,
        "cos_similarities_scaled": cos_similarities_scaled,
        "u_idx": u_idx,
        "i_idx": i_idx,
    }
    in_maps, K, host = _host_prep(inputs)
    nc = build_nc(K)
    res = run_bass_kernel_spmd(nc, in_maps, core_ids=list(range(NCORES)), trace=_trace)
    loss = _combine([res.results[c] for c in range(NCORES)], host)
    if _trace:
        kernel._last_results = res
    return np.float32(loss)


def _combine(outs_per_core, host):
    """Host-side glue: sum colsum partials, ln+dot with rowR, add exact T/MSE."""
    row_r = host["row_r"]  # fp64 [N]
    colsum = np.zeros(M, dtype=np.float64)
    S2 = 0.0
    PAIR = 0.0
    for c in range(NCORES):
        o = outs_per_core[c]
        colsum += np.asarray(o["out_cs"], dtype=np.float64)
        rs = np.asarray(o["out_rs"], dtype=np.float64)      # [128, RT]: user r*128+p
        rows = rs.T.reshape(-1)                              # user index within slab
        S2 += float(row_r[c * S:(c + 1) * S] @ np.log(rows))
        PAIR += float(np.asarray(o["out_sc"], dtype=np.float64).sum())
    S3 = float(row_r @ np.log(colsum))
    S1 = FILL * host["T"] + PAIR
    contrastive = (S2 + S3 - 2.0 * S1) / (2.0 * N)
    return ALPHA * contrastive + (1.0 - ALPHA) * host["mse"]


# revision 9
# speedup vs baseline: 1.4606x; 1.0033x over previous
"""Trainium2 Bass kernel for nn_ContrastiveMSELoss (8192x8192 cos-sim contrastive + MSE).

Sharding: 8 NeuronCores, users row-sharded 1024/core, full recipe table per core.

Loss decomposition (ratings matrix never materialized):
    rowR[i]  = 0.1*M + sum_{final scatter cells in row i}(v - 0.1)
    S1       = 0.1*T + sum_pairs (v-0.1)*cos[u,i],  T = (sum_i u_i/|u_i|) . (sum_j r_j/|r_j|)
    S2       = sum_i rowR[i] * ln(rowsum_exp[i])
    S3       = sum_i rowR[i] * ln(colsum_exp[i])    (col_sum indexed by i: torch n==m quirk)
    loss     = 0.5*(S2 + S3 - 2*S1)/(2*N) + 0.5*mean((ratings-cos_sim)^2)

Device computes, per core: the 1024x8192 exp(cos) block via fp8e4 DoubleRow
matmuls (host ships pre-normalized, pre-transposed fp8 operands), exp split
across three lanes (ACT exp / Pool and DVE Schraudolph bit-trick exp in fp16),
per-row sums (DVE tensor_scalar accumulate), per-column partial sums (PE
ones-matmul chains into 4 PSUM tiles x 4 tile positions), and the scattered
pair dot products (Pool dma_gather of bf16 rows + DVE dots).

No collective: each core ships its rowsums [1024], colsum partials [8192] and
pair-term partial [128]; the host does the O(N) ln/dot glue (exactly like the
scalar summing it already does), plus the exact closed-form T and MSE terms.
"""

import sys

sys.path.insert(0, "/opt/trn_rl_repo")

import numpy as np
import ml_dtypes

import concourse.bass as bass
import concourse.bacc as bacc
import concourse.tile as tile
from concourse import mybir
from concourse.bass_utils import run_bass_kernel_spmd

f32 = mybir.dt.float32
bf16 = mybir.dt.bfloat16
fp16 = mybir.dt.float16
fp8 = mybir.dt.float8e4
i16 = mybir.dt.int16
AF = mybir.ActivationFunctionType
OP = mybir.AluOpType
AX = mybir.AxisListType
PM = mybir.MatmulPerfMode

NCORES = 8
N = 8192          # users
M = 8192          # recipes
D = 64
B = 65536
S = N // NCORES   # slab rows per core (1024)
RT = S // 128     # row tiles per slab (8)
NG = 8            # column groups of 1024
ALPHA = 0.5
FILL = 0.1
GATHER_CHUNK = 512

# Schraudolph fast-exp in fp16-bit domain: i16 = round(A*x + B), bitcast fp16.
SCH_A = 1024.0 / float(np.log(2.0))   # 1477.32
SCH_B = 15.0 * 1024.0 - 40.0          # c=40 zeroes the exp-sum bias for cos~N(0,1/64)

# exp lane assignment: ACT / POOL weighted round-robin over the 64 tiles
# (single-op Schraudolph on Pool: TSP fp32->int16, bitcast fp16)
LANE_W = {"A": 37, "P": 27}
# groups whose colsum runs as DVE fp16 accumulate + one final ones-matmul
# (relieves PE, which otherwise pays 426ns/tile for ones-matmul chains)
DVE_CS_GROUPS = (2, 5)


def _lane_list():
    lanes = []
    cnt = {k: 0 for k in LANE_W}
    for i in range(64):
        best, bestv = None, None
        for k, w in LANE_W.items():
            v = w * (i + 1) / 64.0 - cnt[k]
            if bestv is None or v > bestv:
                best, bestv = k, v
        lanes.append(best)
        cnt[best] += 1
    return lanes


def build_nc(K):
    """Build the SPMD Bass program. K = pair slots per partition (128*K pairs/core)."""
    nc = bacc.Bacc(num_devices=NCORES)
    NP = 128 * K

    u8_d = nc.declare_dram_parameter("u8", [32, 2, S], fp8, isOutput=False)
    r8_d = nc.declare_dram_parameter("r8", [32, 2, M], fp8, isOutput=False)
    uhat_d = nc.declare_dram_parameter("uhat", [S, 2 * D], bf16, isOutput=False)
    rhat_d = nc.declare_dram_parameter("rhat", [M, 2 * D], bf16, isOutput=False)
    pair_u = nc.declare_dram_parameter("pair_u", [128, NP // 16], i16, isOutput=False)
    pair_i = nc.declare_dram_parameter("pair_i", [128, NP // 16], i16, isOutput=False)
    pair_w = nc.declare_dram_parameter("pair_w", [128, K], f32, isOutput=False)
    out_cs = nc.declare_dram_parameter("out_cs", [M], f32, isOutput=True)
    out_rs = nc.declare_dram_parameter("out_rs", [128, RT], f32, isOutput=True)
    out_sc = nc.declare_dram_parameter("out_sc", [128, 1], f32, isOutput=True)

    lanes = _lane_list()

    with tile.TileContext(nc) as tc:
        with tc.tile_pool(name="sb", bufs=1) as sb, \
             tc.tile_pool(name="ps", bufs=1, space="PSUM") as ps:
            # ---- input loads ----
            u8s = sb.tile([32, 2, S], fp8)
            nc.sync.dma_start(out=u8s[:], in_=u8_d[:])
            r8g = []
            for g in range(NG):
                t = sb.tile([32, 2, 1024], fp8, name=f"r8g{g}")
                nc.sync.dma_start(out=t[:], in_=r8_d[:, :, g * 1024:(g + 1) * 1024])
                r8g.append(t)
            pu = sb.tile([128, NP // 16], i16)
            nc.sync.dma_start(out=pu[:], in_=pair_u[:])
            pi = sb.tile([128, NP // 16], i16)
            nc.sync.dma_start(out=pi[:], in_=pair_i[:])
            pw = sb.tile([128, K], f32)
            nc.sync.dma_start(out=pw[:], in_=pair_w[:])

            ones16 = sb.tile([128, 1], fp16)
            nc.vector.memset(ones16[:], 1.0)

            # pair gathers (Pool SWDGE): issued in chunks interleaved into the
            # main loop so they don't monopolize the in-order Pool queue.
            ug = sb.tile([128, K, 2 * D], bf16)
            rg = sb.tile([128, K, 2 * D], bf16)
            n_chunks = (NP + GATHER_CHUNK - 1) // GATHER_CHUNK  # per tensor

            def issue_gathers(tensor_i, lo, hi):
                dst, src, idx = [(ug, uhat_d, pu), (rg, rhat_d, pi)][tensor_i]
                for ci in range(lo, min(hi, n_chunks)):
                    off = ci * GATHER_CHUNK
                    n = min(GATHER_CHUNK, NP - off)
                    nc.gpsimd.dma_gather(
                        dst[:, off // 128:(off + n) // 128, :], src[:],
                        idx[:, off // 16:(off + n) // 16], n, n, 2 * D)

            rs_acc = sb.tile([128, NG * RT], f32)
            junk = sb.tile([128, 1024], fp16)

            # pair-dot block state (issued in 4 chunks across late groups)
            KC = (K + 3) // 4
            cosg = sb.tile([128, K], f32)
            pair_acc = sb.tile([128, 4], f32)
            nc.vector.memset(pair_acc[:], 0.0)

            def issue_pair_chunk(c):
                ks = slice(c * KC, min((c + 1) * KC, K))
                nk = ks.stop - ks.start
                if nk <= 0:
                    return
                x_t = sb.tile([128, KC, D], bf16, tag="pairx", bufs=2)
                nc.vector.tensor_tensor(
                    out=x_t[:, 0:nk, :], in0=ug[:, ks, 0:D], in1=rg[:, ks, 0:D], op=OP.mult)
                nc.vector.tensor_reduce(
                    out=cosg[:, ks], in_=x_t[:, 0:nk, :], axis=AX.X, op=OP.add)
                junk_k = sb.tile([128, KC], f32, tag="pairj", bufs=2)
                nc.vector.tensor_tensor_reduce(
                    out=junk_k[:, 0:nk], in0=cosg[:, ks], in1=pw[:, ks], scale=1.0,
                    scalar=0.0, op0=OP.mult, op1=OP.add,
                    accum_out=pair_acc[:, c:c + 1])

            # ---- main loop ----
            li = 0
            for g in range(NG):
                if g % 2 == 0:
                    cs_t = ps.tile([128, 512], f32, tag="cs", bufs=2)
                dve_cs = g in DVE_CS_GROUPS
                if dve_cs:
                    acc_g = sb.tile([128, 1024], fp16, tag="accg", bufs=2)
                for r in range(RT):
                    pg = ps.tile([128, 1024], f32, tag="cos", bufs=3)
                    for jj in range(2):
                        nc.tensor.matmul(
                            out=pg[:, jj * 512:(jj + 1) * 512],
                            lhsT=u8s[:, :, r * 128:(r + 1) * 128],
                            rhs=r8g[g][:, :, jj * 512:(jj + 1) * 512],
                            start=True, stop=True,
                            perf_mode=PM.DoubleRow)
                    lane = lanes[li]
                    li += 1
                    if lane == "A":
                        ex_t = sb.tile([128, 1024], fp16, tag="ex", bufs=4)
                        nc.scalar.activation(out=ex_t[:], in_=pg[:], func=AF.Exp)
                        ex = ex_t[:]
                    else:
                        ip = sb.tile([128, 1024], i16, tag="ex8", bufs=4)
                        nc.gpsimd.tensor_scalar(out=ip[:], in0=pg[:], scalar1=SCH_A,
                                                scalar2=SCH_B, op0=OP.mult, op1=OP.add)
                        ex = ip[:].bitcast(fp16)
                    # rowsum partial (DVE TSP accumulate)
                    nc.vector.tensor_scalar(
                        out=junk[:], in0=ex, scalar1=1.0, scalar2=None,
                        op0=OP.mult, op1=OP.add,
                        accum_out=rs_acc[:, g * RT + r:g * RT + r + 1])
                    # colsum: DVE fp16 accumulate for the chosen groups,
                    # PE ones-matmul chains (4 positions/PSUM tile) otherwise
                    if dve_cs:
                        if r == 0:
                            nc.vector.tensor_copy(out=acc_g[:], in_=ex)
                        else:
                            nc.vector.tensor_tensor(
                                out=acc_g[:], in0=acc_g[:], in1=ex, op=OP.add)
                        if r == RT - 1:
                            for jj in range(2):
                                pos = (g % 2) * 64 + jj * 32
                                nc.tensor.matmul(
                                    out=cs_t[pos:pos + 1, :],
                                    lhsT=ones16[:, 0:1],
                                    rhs=acc_g[:, jj * 512:(jj + 1) * 512],
                                    start=True, stop=True,
                                    tile_position=(0, pos),
                                    skip_group_check=True)
                    else:
                        for jj in range(2):
                            pos = (g % 2) * 64 + jj * 32
                            nc.tensor.matmul(
                                out=cs_t[pos:pos + 1, :],
                                lhsT=ones16[:, 0:1],
                                rhs=ex[:, jj * 512:(jj + 1) * 512],
                                start=(r == 0), stop=(r == RT - 1),
                                tile_position=(0, pos),
                                skip_group_check=True)
                # after odd group: finalize the cs tile (copy 4 rows, ship)
                if g % 2 == 1:
                    t = g // 2
                    csb = sb.tile([128, 512], f32, tag="csb", bufs=2)
                    nc.vector.tensor_copy(out=csb[0:128:32, :], in_=cs_t[0:128:32, :])
                    for q in range(4):
                        seg = (2 * t + q // 2) * 1024 + (q % 2) * 512
                        nc.sync.dma_start(out=out_cs[seg:seg + 512], in_=csb[32 * q:32 * q + 1, :])
                # interleave gathers (6 chunks/group keeps Pool fed but not clogged)
                if g < 5:
                    lo = g * 3
                    issue_gathers(0, lo, lo + 3)
                    issue_gathers(1, lo, lo + 3)
                elif g == 5:
                    issue_gathers(0, 15, n_chunks)
                    issue_gathers(1, 15, n_chunks)
                # pair-dot chunks on DVE, spread across late groups
                if 4 <= g <= 7:
                    issue_pair_chunk(g - 4)

            # pair partial -> out
            pair_f = sb.tile([128, 1], f32)
            nc.vector.tensor_reduce(out=pair_f[:], in_=pair_acc[:], axis=AX.X, op=OP.add)
            nc.sync.dma_start(out=out_sc[:], in_=pair_f[:])

            # ---- tail: rowsums ----
            rs_f = sb.tile([128, RT], f32)
            nc.vector.tensor_reduce(
                out=rs_f[:],
                in_=rs_acc[:].rearrange("p (g r) -> p r g", g=NG),
                axis=AX.X, op=OP.add)
            nc.sync.dma_start(out=out_rs[:], in_=rs_f[:])
    nc.finalize()
    return nc


def _host_prep(inputs):
    """Normalize embeddings, build fp8/bf16 device operands, dedup+shard pairs."""
    U = np.asarray(inputs["user_embeddings"], dtype=np.float32)
    R = np.asarray(inputs["recipe_embeddings"], dtype=np.float32)
    rat = np.asarray(inputs["ratings_scaled"], dtype=np.float32)
    u = np.asarray(inputs["u_idx"]).astype(np.int64)
    i = np.asarray(inputs["i_idx"]).astype(np.int64)

    un = np.linalg.norm(U.astype(np.float64), axis=1)
    rn = np.linalg.norm(R.astype(np.float64), axis=1)
    uhat = (U / un[:, None]).astype(np.float32)
    rhat = (R / rn[:, None]).astype(np.float32)

    # fp8 transposed layouts: [d(32), t(2), row] with k = 32*t + d
    f8 = ml_dtypes.float8_e4m3
    u8_full = np.stack([uhat[:, 0:32].T, uhat[:, 32:64].T], axis=1).astype(f8)  # [32,2,N]
    r8 = np.ascontiguousarray(np.stack([rhat[:, 0:32].T, rhat[:, 32:64].T], axis=1).astype(f8))

    # bf16 row-major padded to 128 cols (256B rows for dma_gather)
    uhat_pad = np.zeros((N, 2 * D), dtype=ml_dtypes.bfloat16)
    uhat_pad[:, 0:D] = uhat.astype(ml_dtypes.bfloat16)
    rhat_pad = np.zeros((M, 2 * D), dtype=ml_dtypes.bfloat16)
    rhat_pad[:, 0:D] = rhat.astype(ml_dtypes.bfloat16)

    # dedup scatter: last write wins
    cell = u * M + i
    _, idx_rev = np.unique(cell[::-1], return_index=True)
    keep = (B - 1 - idx_rev)
    uu = u[keep].astype(np.int64)
    ii = i[keep].astype(np.int64)
    ww = (rat[keep].astype(np.float64) - FILL)

    delta = np.bincount(uu, weights=ww, minlength=N)
    row_r = (FILL * M + delta)  # fp64 [N]

    core_of = uu // S
    counts = np.bincount(core_of, minlength=NCORES)
    K = max(1, int(np.ceil(counts.max() / 128)))
    cap = 128 * K

    in_maps = []
    for c in range(NCORES):
        sel = core_of == c
        n_c = int(sel.sum())
        pu = np.zeros(cap, dtype=np.int16)
        piv = np.zeros(cap, dtype=np.int16)
        pwv = np.zeros(cap, dtype=np.float32)
        pu[:n_c] = (uu[sel] - c * S).astype(np.int16)
        piv[:n_c] = ii[sel].astype(np.int16)
        pwv[:n_c] = ww[sel].astype(np.float32)
        # dma_gather idx layout: [128, cap//16], row p slot s = idx[s*16 + p%16], tiled 8x
        pu_dev = np.ascontiguousarray(np.tile(pu.reshape(cap // 16, 16).T, (8, 1)))
        pi_dev = np.ascontiguousarray(np.tile(piv.reshape(cap // 16, 16).T, (8, 1)))
        # pair weights in TSP layout [128, K]: slot k of partition p = pair 128*k + p
        pw_dev = np.ascontiguousarray(pwv.reshape(K, 128).T)
        in_maps.append({
            "u8": np.ascontiguousarray(u8_full[:, :, c * S:(c + 1) * S]),
            "r8": r8,
            "uhat": np.ascontiguousarray(uhat_pad[c * S:(c + 1) * S]),
            "rhat": rhat_pad,
            "pair_u": pu_dev,
            "pair_i": pi_dev,
            "pair_w": pw_dev,
        })

    host = {
        "row_r": row_r,
        "T": float(uhat.astype(np.float64).sum(0) @ rhat.astype(np.float64).sum(0)),
        "mse": float(np.mean((rat.astype(np.float64)
                              - np.asarray(inputs["cos_similarities_scaled"], dtype=np.float64)) ** 2)),
    }
    return in_maps, K, host


def kernel(user_embeddings, recipe_embeddings, ratings_scaled, cos_similarities_scaled,
           u_idx, i_idx, _trace=False):
    inputs = {
        "user_embeddings": user_embeddings,
        "recipe_embeddings": recipe_embeddings,
        "ratings_scaled": ratings_scaled,
        "cos_similarities_scaled": cos_similarities_scaled,
        "u_idx": u_idx,
        "i_idx": i_idx,
    }
    in_maps, K, host = _host_prep(inputs)
    nc = build_nc(K)
    res = run_bass_kernel_spmd(nc, in_maps, core_ids=list(range(NCORES)), trace=_trace)
    loss = _combine([res.results[c] for c in range(NCORES)], host)
    if _trace:
        kernel._last_results = res
    return np.float32(loss)


def _combine(outs_per_core, host):
    """Host-side glue: sum colsum partials, ln+dot with rowR, add exact T/MSE."""
    row_r = host["row_r"]  # fp64 [N]
    colsum = np.zeros(M, dtype=np.float64)
    S2 = 0.0
    PAIR = 0.0
    for c in range(NCORES):
        o = outs_per_core[c]
        colsum += np.asarray(o["out_cs"], dtype=np.float64)
        rs = np.asarray(o["out_rs"], dtype=np.float64)      # [128, RT]: user r*128+p
        rows = rs.T.reshape(-1)                              # user index within slab
        S2 += float(row_r[c * S:(c + 1) * S] @ np.log(rows))
        PAIR += float(np.asarray(o["out_sc"], dtype=np.float64).sum())
    S3 = float(row_r @ np.log(colsum))
    S1 = FILL * host["T"] + PAIR
    contrastive = (S2 + S3 - 2.0 * S1) / (2.0 * N)
    return ALPHA * contrastive + (1.0 - ALPHA) * host["mse"]


# revision 11
# speedup vs baseline: 1.5651x; 1.0715x over previous
"""Trainium2 Bass kernel for nn_ContrastiveMSELoss (8192x8192 cos-sim contrastive + MSE).

Sharding: 8 NeuronCores, users row-sharded 1024/core, full recipe table per core.

Loss decomposition (ratings matrix never materialized):
    rowR[i]  = 0.1*M + sum_{final scatter cells in row i}(v - 0.1)
    S1       = 0.1*T + sum_pairs (v-0.1)*cos[u,i],  T = (sum_i u_i/|u_i|) . (sum_j r_j/|r_j|)
    S2       = sum_i rowR[i] * ln(rowsum_exp[i])
    S3       = sum_i rowR[i] * ln(colsum_exp[i])    (col_sum indexed by i: torch n==m quirk)
    loss     = 0.5*(S2 + S3 - 2*S1)/(2*N) + 0.5*mean((ratings-cos_sim)^2)

Device computes, per core: the 1024x8192 exp(cos) block via fp8e4 DoubleRow
matmuls (host ships pre-normalized, pre-transposed fp8 operands), exp split
across three lanes (ACT exp / Pool and DVE Schraudolph bit-trick exp in fp16),
per-row sums (DVE tensor_scalar accumulate), per-column partial sums (PE
ones-matmul chains into 4 PSUM tiles x 4 tile positions), and the scattered
pair dot products (Pool dma_gather of bf16 rows + DVE dots).

No collective: each core ships its rowsums [1024], colsum partials [8192] and
pair-term partial [128]; the host does the O(N) ln/dot glue (exactly like the
scalar summing it already does), plus the exact closed-form T and MSE terms.
"""

import sys

sys.path.insert(0, "/opt/trn_rl_repo")

import numpy as np
import ml_dtypes

import concourse.bass as bass
import concourse.bacc as bacc
import concourse.tile as tile
from concourse import mybir
from concourse.bass_utils import run_bass_kernel_spmd

f32 = mybir.dt.float32
bf16 = mybir.dt.bfloat16
fp16 = mybir.dt.float16
fp8 = mybir.dt.float8e4
i16 = mybir.dt.int16
AF = mybir.ActivationFunctionType
OP = mybir.AluOpType
AX = mybir.AxisListType
PM = mybir.MatmulPerfMode

NCORES = 8
N = 8192          # users
M = 8192          # recipes
D = 64
B = 65536
S = N // NCORES   # slab rows per core (1024)
RT = S // 128     # row tiles per slab (8)
NG = 8            # column groups of 1024
ALPHA = 0.5
FILL = 0.1
GATHER_CHUNK = 512

# Schraudolph fast-exp in fp16-bit domain: i16 = round(A*x + B), bitcast fp16.
SCH_A = 1024.0 / float(np.log(2.0))   # 1477.32
SCH_B = 15.0 * 1024.0 - 40.0          # c=40 zeroes the exp-sum bias for cos~N(0,1/64)

# exp lane assignment: ACT / POOL weighted round-robin over the 64 tiles
# (single-op Schraudolph on Pool: TSP fp32->int16, bitcast fp16)
LANE_W = {"A": 37, "P": 27}
# groups whose colsum runs as DVE fp16 accumulate + one final ones-matmul
# (relieves PE, which otherwise pays 426ns/tile for ones-matmul chains)
DVE_CS_GROUPS = (2, 5)


def _lane_list():
    lanes = []
    cnt = {k: 0 for k in LANE_W}
    for i in range(64):
        best, bestv = None, None
        for k, w in LANE_W.items():
            v = w * (i + 1) / 64.0 - cnt[k]
            if bestv is None or v > bestv:
                best, bestv = k, v
        lanes.append(best)
        cnt[best] += 1
    return lanes


def build_nc(K):
    """Build the SPMD Bass program. K = pair slots per partition (128*K pairs/core)."""
    nc = bacc.Bacc(num_devices=NCORES)
    NP = 128 * K

    u8_d = nc.declare_dram_parameter("u8", [32, 2, S], fp8, isOutput=False)
    r8_d = nc.declare_dram_parameter("r8", [32, 2, M], fp8, isOutput=False)
    uhat_d = nc.declare_dram_parameter("uhat", [S, 2 * D], bf16, isOutput=False)
    rhat_d = nc.declare_dram_parameter("rhat", [M, 2 * D], bf16, isOutput=False)
    pair_u = nc.declare_dram_parameter("pair_u", [128, NP // 16], i16, isOutput=False)
    pair_i = nc.declare_dram_parameter("pair_i", [128, NP // 16], i16, isOutput=False)
    pair_w = nc.declare_dram_parameter("pair_w", [128, K], f32, isOutput=False)
    out_cs = nc.declare_dram_parameter("out_cs", [M], f32, isOutput=True)
    out_rs = nc.declare_dram_parameter("out_rs", [128, RT], f32, isOutput=True)
    out_sc = nc.declare_dram_parameter("out_sc", [128, 1], f32, isOutput=True)

    lanes = _lane_list()

    with tile.TileContext(nc) as tc:
        with tc.tile_pool(name="sb", bufs=1) as sb, \
             tc.tile_pool(name="ps", bufs=1, space="PSUM") as ps:
            # ---- input loads ----
            u8s = sb.tile([32, 2, S], fp8)
            nc.sync.dma_start(out=u8s[:], in_=u8_d[:])
            r8g = []
            for g in range(NG):
                t = sb.tile([32, 2, 1024], fp8, name=f"r8g{g}")
                nc.sync.dma_start(out=t[:], in_=r8_d[:, :, g * 1024:(g + 1) * 1024])
                r8g.append(t)
            pu = sb.tile([128, NP // 16], i16)
            nc.sync.dma_start(out=pu[:], in_=pair_u[:])
            pi = sb.tile([128, NP // 16], i16)
            nc.sync.dma_start(out=pi[:], in_=pair_i[:])
            pw = sb.tile([128, K], f32)
            nc.sync.dma_start(out=pw[:], in_=pair_w[:])

            ones16 = sb.tile([128, 1], fp16)
            nc.vector.memset(ones16[:], 1.0)

            # pair gathers (Pool SWDGE): issued in chunks interleaved into the
            # main loop so they don't monopolize the in-order Pool queue.
            ug = sb.tile([128, K, 2 * D], bf16)
            rg = sb.tile([128, K, 2 * D], bf16)
            n_chunks = (NP + GATHER_CHUNK - 1) // GATHER_CHUNK  # per tensor

            def issue_gathers(tensor_i, lo, hi):
                dst, src, idx = [(ug, uhat_d, pu), (rg, rhat_d, pi)][tensor_i]
                for ci in range(lo, min(hi, n_chunks)):
                    off = ci * GATHER_CHUNK
                    n = min(GATHER_CHUNK, NP - off)
                    nc.gpsimd.dma_gather(
                        dst[:, off // 128:(off + n) // 128, :], src[:],
                        idx[:, off // 16:(off + n) // 16], n, n, 2 * D)

            rs_acc = sb.tile([128, NG * RT], f32)
            junk = sb.tile([128, 1024], fp16)

            # pair-dot block state (issued in 4 chunks across late groups)
            KC = (K + 3) // 4
            cosg = sb.tile([128, K], f32)
            pair_acc = sb.tile([128, 4], f32)
            nc.vector.memset(pair_acc[:], 0.0)

            def issue_pair_chunk(c):
                ks = slice(c * KC, min((c + 1) * KC, K))
                nk = ks.stop - ks.start
                if nk <= 0:
                    return
                x_t = sb.tile([128, KC, D], bf16, tag="pairx", bufs=2)
                nc.vector.tensor_tensor(
                    out=x_t[:, 0:nk, :], in0=ug[:, ks, 0:D], in1=rg[:, ks, 0:D], op=OP.mult)
                nc.vector.tensor_reduce(
                    out=cosg[:, ks], in_=x_t[:, 0:nk, :], axis=AX.X, op=OP.add)
                junk_k = sb.tile([128, KC], f32, tag="pairj", bufs=2)
                nc.vector.tensor_tensor_reduce(
                    out=junk_k[:, 0:nk], in0=cosg[:, ks], in1=pw[:, ks], scale=1.0,
                    scalar=0.0, op0=OP.mult, op1=OP.add,
                    accum_out=pair_acc[:, c:c + 1])

            # ---- main loop ----
            # PE-order decoupling: each tile's colsum ones-matmul waits on that
            # tile's exp; issuing it right before the next tile's cos matmuls
            # convoys PE behind exp latency. Flush colsums LAG tiles late.
            LAG = 4
            cs_for = {}      # even-group -> psum tile
            acc_for = {}     # dve-cs group -> fp16 accumulator
            ex_for = {}      # tile index -> ex AP
            li = 0

            def flush_colsum(i):
                g, r = divmod(i, RT)
                if g % 2 == 0 and r == 0:
                    cs_for[g] = ps.tile([128, 512], f32, tag="cs", bufs=2, name=f"cs{g}")
                cs_t = cs_for[g - g % 2]
                ex = ex_for.pop(i)
                if g in DVE_CS_GROUPS:
                    acc_g = acc_for.get(g)
                    if r == 0:
                        acc_g = acc_for[g] = sb.tile([128, 1024], fp16, tag="accg", bufs=2, name=f"accg{g}")
                        nc.vector.tensor_copy(out=acc_g[:], in_=ex)
                    else:
                        nc.vector.tensor_tensor(
                            out=acc_g[:], in0=acc_g[:], in1=ex, op=OP.add)
                    if r == RT - 1:
                        for jj in range(2):
                            pos = (g % 2) * 64 + jj * 32
                            nc.tensor.matmul(
                                out=cs_t[pos:pos + 1, :],
                                lhsT=ones16[:, 0:1],
                                rhs=acc_g[:, jj * 512:(jj + 1) * 512],
                                start=True, stop=True,
                                tile_position=(0, pos),
                                skip_group_check=True)
                else:
                    for jj in range(2):
                        pos = (g % 2) * 64 + jj * 32
                        nc.tensor.matmul(
                            out=cs_t[pos:pos + 1, :],
                            lhsT=ones16[:, 0:1],
                            rhs=ex[:, jj * 512:(jj + 1) * 512],
                            start=(r == 0), stop=(r == RT - 1),
                            tile_position=(0, pos),
                            skip_group_check=True)
                if g % 2 == 1 and r == RT - 1:
                    # finalize the cs tile pair (copy 4 chain rows, ship)
                    t = g // 2
                    csb = sb.tile([128, 512], f32, tag="csb", bufs=2)
                    nc.vector.tensor_copy(out=csb[0:128:32, :], in_=cs_t[0:128:32, :])
                    for q in range(4):
                        seg = (2 * t + q // 2) * 1024 + (q % 2) * 512
                        nc.sync.dma_start(out=out_cs[seg:seg + 512],
                                          in_=csb[32 * q:32 * q + 1, :])

            NT = NG * RT
            for i in range(NT + LAG):
                if i >= LAG:
                    flush_colsum(i - LAG)
                if i >= NT:
                    continue
                g, r = divmod(i, RT)
                pg = ps.tile([128, 1024], f32, tag="cos", bufs=3)
                for jj in range(2):
                    nc.tensor.matmul(
                        out=pg[:, jj * 512:(jj + 1) * 512],
                        lhsT=u8s[:, :, r * 128:(r + 1) * 128],
                        rhs=r8g[g][:, :, jj * 512:(jj + 1) * 512],
                        start=True, stop=True,
                        perf_mode=PM.DoubleRow)
                lane = lanes[li]
                li += 1
                if lane == "A":
                    ex_t = sb.tile([128, 1024], fp16, tag="ex", bufs=7)
                    nc.scalar.activation(out=ex_t[:], in_=pg[:], func=AF.Exp)
                    ex_for[i] = ex_t[:]
                else:
                    ip = sb.tile([128, 1024], i16, tag="ex8", bufs=7)
                    nc.gpsimd.tensor_scalar(out=ip[:], in0=pg[:], scalar1=SCH_A,
                                            scalar2=SCH_B, op0=OP.mult, op1=OP.add)
                    ex_for[i] = ip[:].bitcast(fp16)
                # rowsum partial (DVE TSP accumulate)
                nc.vector.tensor_scalar(
                    out=junk[:], in0=ex_for[i], scalar1=1.0, scalar2=None,
                    op0=OP.mult, op1=OP.add,
                    accum_out=rs_acc[:, i:i + 1])
                if r == RT - 1:
                    # per-group side work, interleaved at group boundaries
                    if g < 5:
                        issue_gathers(0, g * 3, g * 3 + 3)
                        issue_gathers(1, g * 3, g * 3 + 3)
                    elif g == 5:
                        issue_gathers(0, 15, n_chunks)
                        issue_gathers(1, 15, n_chunks)
                    if 4 <= g <= 7:
                        issue_pair_chunk(g - 4)

            # pair partial -> out
            pair_f = sb.tile([128, 1], f32)
            nc.vector.tensor_reduce(out=pair_f[:], in_=pair_acc[:], axis=AX.X, op=OP.add)
            nc.sync.dma_start(out=out_sc[:], in_=pair_f[:])

            # ---- tail: rowsums ----
            rs_f = sb.tile([128, RT], f32)
            nc.vector.tensor_reduce(
                out=rs_f[:],
                in_=rs_acc[:].rearrange("p (g r) -> p r g", g=NG),
                axis=AX.X, op=OP.add)
            nc.sync.dma_start(out=out_rs[:], in_=rs_f[:])
    nc.finalize()
    return nc


def _host_prep(inputs):
    """Normalize embeddings, build fp8/bf16 device operands, dedup+shard pairs."""
    U = np.asarray(inputs["user_embeddings"], dtype=np.float32)
    R = np.asarray(inputs["recipe_embeddings"], dtype=np.float32)
    rat = np.asarray(inputs["ratings_scaled"], dtype=np.float32)
    u = np.asarray(inputs["u_idx"]).astype(np.int64)
    i = np.asarray(inputs["i_idx"]).astype(np.int64)

    un = np.linalg.norm(U.astype(np.float64), axis=1)
    rn = np.linalg.norm(R.astype(np.float64), axis=1)
    uhat = (U / un[:, None]).astype(np.float32)
    rhat = (R / rn[:, None]).astype(np.float32)

    # fp8 transposed layouts: [d(32), t(2), row] with k = 32*t + d
    f8 = ml_dtypes.float8_e4m3
    u8_full = np.stack([uhat[:, 0:32].T, uhat[:, 32:64].T], axis=1).astype(f8)  # [32,2,N]
    r8 = np.ascontiguousarray(np.stack([rhat[:, 0:32].T, rhat[:, 32:64].T], axis=1).astype(f8))

    # bf16 row-major padded to 128 cols (256B rows for dma_gather)
    uhat_pad = np.zeros((N, 2 * D), dtype=ml_dtypes.bfloat16)
    uhat_pad[:, 0:D] = uhat.astype(ml_dtypes.bfloat16)
    rhat_pad = np.zeros((M, 2 * D), dtype=ml_dtypes.bfloat16)
    rhat_pad[:, 0:D] = rhat.astype(ml_dtypes.bfloat16)

    # dedup scatter: last write wins
    cell = u * M + i
    _, idx_rev = np.unique(cell[::-1], return_index=True)
    keep = (B - 1 - idx_rev)
    uu = u[keep].astype(np.int64)
    ii = i[keep].astype(np.int64)
    ww = (rat[keep].astype(np.float64) - FILL)

    delta = np.bincount(uu, weights=ww, minlength=N)
    row_r = (FILL * M + delta)  # fp64 [N]

    core_of = uu // S
    counts = np.bincount(core_of, minlength=NCORES)
    K = max(1, int(np.ceil(counts.max() / 128)))
    cap = 128 * K

    in_maps = []
    for c in range(NCORES):
        sel = core_of == c
        n_c = int(sel.sum())
        pu = np.zeros(cap, dtype=np.int16)
        piv = np.zeros(cap, dtype=np.int16)
        pwv = np.zeros(cap, dtype=np.float32)
        pu[:n_c] = (uu[sel] - c * S).astype(np.int16)
        piv[:n_c] = ii[sel].astype(np.int16)
        pwv[:n_c] = ww[sel].astype(np.float32)
        # dma_gather idx layout: [128, cap//16], row p slot s = idx[s*16 + p%16], tiled 8x
        pu_dev = np.ascontiguousarray(np.tile(pu.reshape(cap // 16, 16).T, (8, 1)))
        pi_dev = np.ascontiguousarray(np.tile(piv.reshape(cap // 16, 16).T, (8, 1)))
        # pair weights in TSP layout [128, K]: slot k of partition p = pair 128*k + p
        pw_dev = np.ascontiguousarray(pwv.reshape(K, 128).T)
        in_maps.append({
            "u8": np.ascontiguousarray(u8_full[:, :, c * S:(c + 1) * S]),
            "r8": r8,
            "uhat": np.ascontiguousarray(uhat_pad[c * S:(c + 1) * S]),
            "rhat": rhat_pad,
            "pair_u": pu_dev,
            "pair_i": pi_dev,
            "pair_w": pw_dev,
        })

    host = {
        "row_r": row_r,
        "T": float(uhat.astype(np.float64).sum(0) @ rhat.astype(np.float64).sum(0)),
        "mse": float(np.mean((rat.astype(np.float64)
                              - np.asarray(inputs["cos_similarities_scaled"], dtype=np.float64)) ** 2)),
    }
    return in_maps, K, host


def kernel(user_embeddings, recipe_embeddings, ratings_scaled, cos_similarities_scaled,
           u_idx, i_idx, _trace=False):
    inputs = {
        "user_embeddings": user_embeddings,
        "recipe_embeddings": recipe_embeddings,
        "ratings_scaled": ratings_scaled,
        "cos_similarities_scaled": cos_similarities_scaled,
        "u_idx": u_idx,
        "i_idx": i_idx,
    }
    in_maps, K, host = _host_prep(inputs)
    nc = build_nc(K)
    res = run_bass_kernel_spmd(nc, in_maps, core_ids=list(range(NCORES)), trace=_trace)
    loss = _combine([res.results[c] for c in range(NCORES)], host)
    if _trace:
        kernel._last_results = res
    return np.float32(loss)


def _combine(outs_per_core, host):
    """Host-side glue: sum colsum partials, ln+dot with rowR, add exact T/MSE."""
    row_r = host["row_r"]  # fp64 [N]
    colsum = np.zeros(M, dtype=np.float64)
    S2 = 0.0
    PAIR = 0.0
    for c in range(NCORES):
        o = outs_per_core[c]
        colsum += np.asarray(o["out_cs"], dtype=np.float64)
        rs = np.asarray(o["out_rs"], dtype=np.float64)      # [128, RT]: user r*128+p
        rows = rs.T.reshape(-1)                              # user index within slab
        S2 += float(row_r[c * S:(c + 1) * S] @ np.log(rows))
        PAIR += float(np.asarray(o["out_sc"], dtype=np.float64).sum())
    S3 = float(row_r @ np.log(colsum))
    S1 = FILL * host["T"] + PAIR
    contrastive = (S2 + S3 - 2.0 * S1) / (2.0 * N)
    return ALPHA * contrastive + (1.0 - ALPHA) * host["mse"]
